# revision 1
# baseline (speedup 1.0000x reference)
"""CrossTypeHGNN Trainium2 kernel.

Reference computation (per node type i in {0,1,2}, N=6144, F=64):
    u_i = sum_{j != i} H_ij @ x_j              # layer-1 cross-type aggregation
    h_i = u_i @ W1_i.T + b1_i
    v_i = sum_{j != i} H_ij @ h_j              # layer-2 on hidden features
    out_i = v_i @ W2_i.T + b2_i

Strategy (8 NeuronCores):
  - Row-shard every H_ij across cores (768 rows each).  The shard is shipped
    HOST-TRANSPOSED and bf16-cast as ht[t, p, m, r] = H_m[768*core + r, 128*t + p],
    so on-device the contraction dim (H columns) is already the SBUF partition
    dim: no device transposes, and half the DMA bytes (memory-bound problem).
  - Layer 1: psum[f, r] += x_j[c-tile].T @ Ht[c-tile, r]  (x stationary 128x64
    bf16, Ht moving 128x384) accumulated over 48 c-tiles; all 6 H matrices
    share one interleaved DMA per c-tile.  Layer 1 streams H in fp8-e4m3
    (values pre-scaled by N on the host, 1/N folded into W1; mixed
    bf16-stationary x fp8-moving matmul is supported by the PE), halving
    layer-1 H bytes; layer 2 streams H in bf16.  Final absmax-rel error vs
    the fp32 reference is ~6e-5 either way, because h is bias-dominated and
    the layer-2 aggregation averages layer-1 quantization noise away.
  - Tiny 64x64 linears run in fp32 from pre-transposed W shipped by the host;
    bias is a per-partition tensor_scalar_add that also evicts PSUM.
  - h.T -> h via PE transpose, AllGather (bf16) across the 8 cores, layer 2
    mirrors layer 1 with h as the stationary operand, H re-streamed from DRAM.
  - Outputs stay transposed ([3, 64, 768] per core); host transposes/concats.
"""

import numpy as np
import ml_dtypes
from contextlib import ExitStack

import concourse.bacc as bacc
import concourse.mybir as mybir
import concourse.tile as tile
from concourse.bass_utils import run_bass_kernel_spmd
from concourse.masks import make_identity

N = 6144
F = 64
CORES = 8
R = N // CORES            # 768 rows per core
T = N // 128              # 48 contraction tiles
LT = R // 128             # 6 local row tiles
NH = 384                  # psum half of the 768-wide free dim (one bank)

PAIRS = [(0, 1), (0, 2), (1, 0), (1, 2), (2, 0), (2, 1)]  # m -> (i, j)
# within a c-tile, visit matrices grouped by j so consecutive matmuls share
# the stationary x_j / h_j tile
M_ORDER = [0, 5, 1, 3, 2, 4]
FIRST_M = {0: 0, 2: 5, 1: 3}  # first m in M_ORDER emitting into acc[i]
LAST_M = {0: 1, 1: 2, 2: 4}   # last m in M_ORDER emitting into acc[i]

BF16 = mybir.dt.bfloat16
F8 = mybir.dt.float8e4
F32 = mybir.dt.float32


def build_module(n_repeats=1):
    """n_repeats > 1 repeats the full compute inside one NEFF; used by the
    timing harness to measure marginal per-iteration HW time (cancels axon
    dispatch + per-call input staging)."""
    nc = bacc.Bacc("TRN2", target_bir_lowering=False, debug=False, num_devices=CORES)

    # layer 1 streams H in fp8-e4m3 (values pre-scaled by N on the host; the
    # 1/N is folded into W1), with the bf16 x as the stationary operand —
    # mixed-dtype matmul is supported and exact for these operands.  Layer 2
    # re-streams H in bf16.  This cuts total H DMA bytes by 25%.
    ht8_d = nc.dram_tensor("ht8", [T, 128, 6, R], F8, kind="ExternalInput")
    ht_d = nc.dram_tensor("ht", [T, 128, 6, R], BF16, kind="ExternalInput")
    xt_d = nc.dram_tensor("xt", [128, 3, T, F], BF16, kind="ExternalInput")
    w1t_d = nc.dram_tensor("w1t", [F, 3, F], F32, kind="ExternalInput")
    w2t_d = nc.dram_tensor("w2t", [F, 3, F], F32, kind="ExternalInput")
    b1_d = nc.dram_tensor("b1", [F, 3, 1], F32, kind="ExternalInput")
    b2_d = nc.dram_tensor("b2", [F, 3, 1], F32, kind="ExternalInput")
    outT_d = nc.dram_tensor("outT", [3, F, R], F32, kind="ExternalOutput")

    with tile.TileContext(nc) as tc, ExitStack() as ctx:
        const = ctx.enter_context(tc.tile_pool(name="const", bufs=1))
        # ht8 (fp8, layer 1) needs little depth — layer 1 is PE-bound; ht
        # (bf16, layer 2) gets deep buffering so layer-2 tiles prefetch during
        # layer 1's spare DMA capacity.
        htp = ctx.enter_context(tc.tile_pool(name="htp", bufs=6))
        work = ctx.enter_context(tc.tile_pool(name="work", bufs=2))
        pacc = ctx.enter_context(tc.tile_pool(name="pacc", bufs=6, space="PSUM"))
        pmisc = ctx.enter_context(tc.tile_pool(name="pmisc", bufs=2, space="PSUM"))
        dram = ctx.enter_context(tc.tile_pool(name="dram", bufs=1, space="DRAM"))

        # ---- constants -----------------------------------------------------
        x_sb = const.tile([128, 3, T, F], BF16)
        nc.sync.dma_start(x_sb[:], xt_d[:])
        w1_sb = const.tile([F, 3, F], F32)
        nc.sync.dma_start(w1_sb[:], w1t_d[:])
        w2_sb = const.tile([F, 3, F], F32)
        nc.sync.dma_start(w2_sb[:], w2t_d[:])
        b1_sb = const.tile([F, 3, 1], F32)
        nc.sync.dma_start(b1_sb[:], b1_d[:])
        b2_sb = const.tile([F, 3, 1], F32)
        nc.sync.dma_start(b2_sb[:], b2_d[:])
        identity = const.tile([128, 128], BF16)
        make_identity(nc, identity)

        h_sb = const.tile([128, 3, T, F], BF16)  # layer-2 stationary (post-AG)

        ag_tiles = []
        for _rep in range(n_repeats):
            ag_in = dram.tile([3, R, F], BF16, name=f"ag_in_{_rep}", tag=f"agi{_rep}")
            ag_out = dram.tile(
                [CORES, 3, R, F], BF16, addr_space="Shared",
                name=f"ag_out_{_rep}", tag=f"ago{_rep}",
            )
            ag_tiles.append((ag_in, ag_out))

        # ---- one layer: aggregation matmuls + per-type linear --------------
        def layer(lnum, ag_in=None):
            stat_sb = x_sb if lnum == 0 else h_sb
            w_sb = w1_sb if lnum == 0 else w2_sb
            b_sb = b1_sb if lnum == 0 else b2_sb

            acc = [
                [
                    pacc.tile([F, NH], F32, name=f"acc{lnum}_{i}_{hh}", tag="acc")
                    for hh in (0, 1)
                ]
                for i in range(3)
            ]
            for pos, t in enumerate(range(T)):
                if lnum == 0:
                    ht_t = htp.tile([128, 6, R], F8, name="ht8_t", tag="ht8", bufs=5)
                    nc.sync.dma_start(ht_t[:], ht8_d[t])
                else:
                    ht_t = htp.tile([128, 6, R], BF16, name="ht_t", tag="ht", bufs=10)
                    nc.sync.dma_start(ht_t[:], ht_d[t])
                for m in M_ORDER:
                    i, j = PAIRS[m]
                    stat = stat_sb[:, j, t, :]
                    st = pos == 0 and m == FIRST_M[i]
                    sp = pos == T - 1 and m == LAST_M[i]
                    for hh in (0, 1):
                        nc.tensor.matmul(
                            acc[i][hh][:],
                            stat,
                            ht_t[:, m, hh * NH : (hh + 1) * NH],
                            start=st,
                            stop=sp,
                        )

            for i in range(3):
                u_sb = work.tile([F, R], F32, name=f"u{lnum}_{i}", tag="u")
                nc.vector.tensor_copy(u_sb[:, 0:NH], acc[i][0][:])
                nc.vector.tensor_copy(u_sb[:, NH:R], acc[i][1][:])
                if lnum == 0:
                    dst = work.tile([F, R], BF16, name=f"hT_{i}", tag="hT")
                else:
                    dst = work.tile([F, R], F32, name=f"oT_{i}", tag="oT")
                for hh in (0, 1):
                    lps = pmisc.tile(
                        [F, NH], F32, name=f"lin{lnum}_{i}_{hh}", tag="misc"
                    )
                    nc.tensor.matmul(
                        lps[:],
                        w_sb[:, i, :],
                        u_sb[:, hh * NH : (hh + 1) * NH],
                        start=True,
                        stop=True,
                    )
                    nc.vector.tensor_scalar_add(
                        dst[:, hh * NH : (hh + 1) * NH], lps[:], b_sb[:, i, :]
                    )
                if lnum == 0:
                    h_nat = work.tile([128, LT, F], BF16, name=f"hnat_{i}", tag="hnat")
                    for lt in range(LT):
                        tp = pmisc.tile([128, F], BF16, name=f"tp{i}_{lt}", tag="misc")
                        nc.tensor.transpose(
                            tp[:], dst[:, lt * 128 : (lt + 1) * 128], identity[0:F, 0:F]
                        )
                        nc.vector.tensor_copy(h_nat[:, lt, :], tp[:])
                    nc.sync.dma_start(
                        ag_in[i].rearrange("(lt p) f -> p lt f", p=128), h_nat[:]
                    )
                else:
                    nc.sync.dma_start(outT_d[i], dst[:])

        for _rep in range(n_repeats):
            ag_in, ag_out = ag_tiles[_rep]
            layer(0, ag_in=ag_in)

            nc.gpsimd.collective_compute(
                "AllGather",
                mybir.AluOpType.bypass,
                replica_groups=[list(range(CORES))],
                ins=[ag_in[:]],
                outs=[ag_out[:]],
            )
            for j in range(3):
                for rank in range(CORES):
                    nc.sync.dma_start(
                        h_sb[:, j, rank * LT : (rank + 1) * LT, :],
                        ag_out[rank, j].rearrange("(lt p) f -> p lt f", p=128),
                    )

            layer(1)

    nc.compile()
    return nc


def prep_inputs(inputs):
    """Host-side shard/transpose/cast. Returns per-core input maps."""
    bf16 = ml_dtypes.bfloat16

    fp8 = ml_dtypes.float8_e4m3

    ht_all = np.empty((CORES, T, 128, 6, R), dtype=bf16)
    ht8_all = np.empty((CORES, T, 128, 6, R), dtype=fp8)
    for m, (i, j) in enumerate(PAIRS):
        Hm = np.asarray(inputs[f"H{i}{j}"], dtype=np.float32)
        # ht_all[core, t, p, m, r] = H[768*core + r, 128*t + p]
        perm = Hm.reshape(CORES, R, T, 128).transpose(0, 2, 3, 1)
        ht_all[:, :, :, m, :] = perm.astype(bf16)
        # layer-1 copy: fp8 with xN rescale (1/N folded into W1 below)
        ht8_all[:, :, :, m, :] = (perm * np.float32(N)).astype(fp8)

    xt = np.empty((128, 3, T, F), dtype=bf16)
    for jj in range(3):
        xj = np.asarray(inputs[f"x{jj}"], dtype=np.float32).astype(bf16)
        xt[:, jj, :, :] = xj.reshape(T, 128, F).transpose(1, 0, 2)

    def stack_wt(key, scale=1.0):
        # [k, 3, o] with w[k, i, o] = scale * W_i[o, k]
        return np.ascontiguousarray(
            np.stack(
                [
                    np.asarray(inputs[f"{key}_{i}"], dtype=np.float32).T
                    * np.float32(scale)
                    for i in range(3)
                ],
                axis=1,
            )
        )

    def stack_b(key):
        return np.ascontiguousarray(
            np.stack(
                [
                    np.asarray(inputs[f"{key}_{i}"], dtype=np.float32).reshape(F, 1)
                    for i in range(3)
                ],
                axis=1,
            )
        )

    shared = {
        "xt": xt,
        "w1t": stack_wt("W1", scale=1.0 / N),  # undo the xN fp8 rescale of H
        "w2t": stack_wt("W2"),
        "b1": stack_b("b1"),
        "b2": stack_b("b2"),
    }
    return [
        {
            "ht": np.ascontiguousarray(ht_all[c]),
            "ht8": np.ascontiguousarray(ht8_all[c]),
            **shared,
        }
        for c in range(CORES)
    ]


_CACHED_NC = None


def get_module():
    global _CACHED_NC
    if _CACHED_NC is None:
        _CACHED_NC = build_module()
    return _CACHED_NC


def kernel(**inputs):
    import time

    nc = get_module()
    in_maps = prep_inputs(inputs)
    last_exc = None
    for attempt in range(3):
        try:
            res = run_bass_kernel_spmd(nc, in_maps, core_ids=list(range(CORES)))
            break
        except Exception as exc:  # transient NRT device errors observed on axon
            last_exc = exc
            time.sleep(5.0)
    else:
        raise last_exc
    outs = []
    for i in range(3):
        outs.append(
            np.ascontiguousarray(
                np.concatenate(
                    [res.results[c]["outT"][i].T for c in range(CORES)], axis=0
                ),
                dtype=np.float32,
            )
        )
    return tuple(outs)


if __name__ == "__main__":
    rng = np.random.default_rng(0)
    inputs = {}
    for i in range(3):
        inputs[f"x{i}"] = rng.standard_normal((N, F), dtype=np.float32)
    for i, j in PAIRS:
        inputs[f"H{i}{j}"] = rng.random((N, N), dtype=np.float32) / N
    for i in range(3):
        inputs[f"W1_{i}"] = rng.standard_normal((F, F), dtype=np.float32) * 0.05
        inputs[f"b1_{i}"] = rng.standard_normal((F,), dtype=np.float32) * 0.05
        inputs[f"W2_{i}"] = rng.standard_normal((F, F), dtype=np.float32) * 0.05
        inputs[f"b2_{i}"] = rng.standard_normal((F,), dtype=np.float32) * 0.05

    out = kernel(**inputs)

    # numpy reference
    def ref(inp):
        u = [None] * 3
        u[0] = inp["H01"] @ inp["x1"] + inp["H02"] @ inp["x2"]
        u[1] = inp["H10"] @ inp["x0"] + inp["H12"] @ inp["x2"]
        u[2] = inp["H20"] @ inp["x0"] + inp["H21"] @ inp["x1"]
        h = [u[i] @ inp[f"W1_{i}"].T + inp[f"b1_{i}"] for i in range(3)]
        v = [None] * 3
        v[0] = inp["H01"] @ h[1] + inp["H02"] @ h[2]
        v[1] = inp["H10"] @ h[0] + inp["H12"] @ h[2]
        v[2] = inp["H20"] @ h[0] + inp["H21"] @ h[1]
        return tuple(v[i] @ inp[f"W2_{i}"].T + inp[f"b2_{i}"] for i in range(3))

    exp = ref(inputs)
    for i in range(3):
        a, e = out[i], exp[i]
        rel = np.abs(a - e).max() / np.abs(e).max()
        print(f"out{i}: absmax-rel err {rel:.3e}")



# revision 3
# speedup vs baseline: 1.6322x; 1.6322x over previous
"""CrossTypeHGNN Trainium2 kernel (v2: fp8 DoubleRow + cross-layer SBUF cache).

Reference computation (per node type i in {0,1,2}, N=6144, F=64):
    u_i = sum_{j != i} H_ij @ x_j              # layer-1 cross-type aggregation
    h_i = u_i @ W1_i.T + b1_i
    v_i = sum_{j != i} H_ij @ h_j              # layer-2 on hidden features
    out_i = v_i @ W2_i.T + b2_i

The problem is HBM-bound (~358 GB/s/core); the only lever is bytes.

Strategy (8 NeuronCores, row-shard every H_ij -> 768 rows/core):
  - H is shipped host-transposed, x(N)-scaled, fp8-e4m3:
        ht8[m, t2, p, k, r] = N * H_m[768*core + r, 256*t2 + 128*k + p]
    One fp8 read of H per core per iteration is the mandatory 28.3 MB;
    a cross-layer SBUF cache holds CACHE_Q of the 24 t2-slices per matrix, so
    layer 2 re-streams only the rest (~12 MB) instead of all of H again.
  - All aggregation matmuls run in MatmulPerfMode.DoubleRow (contraction 256
    rows/instr, 2 fp8 mult/cell/cycle) so the PE keeps pace with DMA.  Both
    operands must be fp8: x is cast to fp8 (decorrelated quantization noise,
    harmless), and the hidden features are passed BIAS-FREE:
        h'_j = (N*u_j) @ W1_j.T          (sigma ~ 18, ideal fp8 range)
    The bias term of layer 2, H_ij @ (1 b1_j^T), is exact:  ones columns
    appended to the layer-1 stationary x make the same matmuls emit
    s_ij = rowsum(N*H_ij-fp8) into psum rows 64..66, and extended W2 rows add
    s_ij * (b1_j @ W2_i.T)/N during the layer-2 linear.
  - h' is AllGather'd per type (3 small fp8 collectives issued as soon as each
    type finishes in layer 1) and scattered into the layer-2 stationary tile;
    layer 2 runs in 3 phases (one per gathered type j), each phase feeding the
    psum accumulators of its two output types.
  - Tiny 64-wide linears run in fp32; outputs leave transposed bf16
    ([3, 64, 768] per core); the host upcasts/transposes/concats.
"""

import numpy as np
import ml_dtypes
from contextlib import ExitStack

import concourse.bacc as bacc
import concourse.mybir as mybir
import concourse.tile as tile
from concourse.bass_utils import run_bass_kernel_spmd
from concourse.masks import make_identity

N = 6144
F = 64
CORES = 8
R = N // CORES            # 768 rows per core
T2 = N // 256             # 24 double-contraction tiles (256 rows each)
LT = R // 128             # 6 local 128-row blocks
NH = 384                  # psum half of the 768-wide free dim (one bank)

PAIRS = [(0, 1), (0, 2), (1, 0), (1, 2), (2, 0), (2, 1)]  # m -> (i, j)
# layer-2 phase j uses the two matrices H_ij with that source type j
PHASE_MS = {0: [2, 4], 1: [0, 5], 2: [1, 3]}
# per output type i: the two phases (j1, j2) that feed acc2[i]
L2_JS = {0: (1, 2), 1: (0, 2), 2: (0, 1)}

CACHE_Q = 14              # t2-slices cached in SBUF per matrix (of 24)

BF16 = mybir.dt.bfloat16
F8 = mybir.dt.float8e4
F32 = mybir.dt.float32
DR = mybir.MatmulPerfMode.DoubleRow


def _cached(m, t2):
    return t2 < CACHE_Q


def build_module(n_repeats=1):
    """n_repeats > 1 repeats the full compute inside one NEFF; used by the
    timing harness to measure marginal per-iteration HW time (cancels axon
    dispatch + per-call input staging).  Every repeat re-streams all inputs."""
    nc = bacc.Bacc("TRN2", target_bir_lowering=False, debug=False, num_devices=CORES)

    ht8_d = nc.dram_tensor("ht8", [6, T2, 128, 2, R], F8, kind="ExternalInput")
    xt8_d = nc.dram_tensor("xt8", [128, 3, 2, T2, 68], F8, kind="ExternalInput")
    w1t_d = nc.dram_tensor("w1t", [F, 3, F], F32, kind="ExternalInput")
    w2e_d = nc.dram_tensor("w2e", [67, 3, F], F32, kind="ExternalInput")
    b2_d = nc.dram_tensor("b2", [F, 3, 1], F32, kind="ExternalInput")
    outT_d = nc.dram_tensor("outT", [3, F, R], BF16, kind="ExternalOutput")

    with tile.TileContext(nc) as tc, ExitStack() as ctx:
        const = ctx.enter_context(tc.tile_pool(name="const", bufs=1))
        xpool = ctx.enter_context(tc.tile_pool(name="xpool", bufs=2))
        cache = ctx.enter_context(tc.tile_pool(name="cache", bufs=1))
        hstream = ctx.enter_context(tc.tile_pool(name="hstream", bufs=6))
        work = ctx.enter_context(tc.tile_pool(name="work", bufs=2))
        pacc = ctx.enter_context(tc.tile_pool(name="pacc", bufs=6, space="PSUM"))
        pmisc = ctx.enter_context(tc.tile_pool(name="pmisc", bufs=2, space="PSUM"))
        dram = ctx.enter_context(tc.tile_pool(name="dram", bufs=1, space="DRAM"))

        # ---- persistent constants ------------------------------------------
        w1_sb = const.tile([F, 3, F], F32)
        nc.sync.dma_start(w1_sb[:], w1t_d[:])
        w2_sb = const.tile([67, 3, F], F32)
        nc.sync.dma_start(w2_sb[:], w2e_d[:])
        b2_sb = const.tile([F, 3, 1], F32)
        nc.sync.dma_start(b2_sb[:], b2_d[:])
        identity = const.tile([128, 128], BF16)
        make_identity(nc, identity)

        # layer-2 stationary: gathered h' in DoubleRow layout, per source type
        S = const.tile([128, 3, 2, T2, F], F8)

        # cross-layer H cache: persistent per-(m, t2) slices
        cache_tiles = {}
        for m in range(6):
            for t2 in range(T2):
                if _cached(m, t2):
                    cache_tiles[(m, t2)] = cache.tile(
                        [128, 2, R], F8, name=f"hc_{m}_{t2}", tag=f"hc_{m}_{t2}"
                    )

        def moving_tile(m, t2, phase):
            """SBUF tile holding ht8[m, t2]; DMA'd here unless cached (layer 2)."""
            if (m, t2) in cache_tiles:
                mt = cache_tiles[(m, t2)]
                if phase == 0:  # layer 1 loads the cache
                    nc.sync.dma_start(mt[:], ht8_d[m, t2])
                return mt
            mt = hstream.tile([128, 2, R], F8, name="hs", tag="hs")
            nc.sync.dma_start(mt[:], ht8_d[m, t2])
            return mt

        for rep in range(n_repeats):
            xt_sb = xpool.tile([128, 3, 2, T2, 68], F8, name=f"xt_{rep}", tag="xt")
            nc.sync.dma_start(xt_sb[:], xt8_d[:])
            s_all = work.tile([67, 3, R], BF16, name=f"sall_{rep}", tag="sall")

            ag_in, ag_out = [], []
            for j in range(3):
                ag_in.append(
                    dram.tile([2, 128, LT // 2, F], F8,
                              name=f"agi_{rep}_{j}", tag=f"agi{rep}_{j}")
                )
                ag_out.append(
                    dram.tile([CORES, 2, 128, LT // 2, F], F8, addr_space="Shared",
                              name=f"ago_{rep}_{j}", tag=f"ago{rep}_{j}")
                )

            # ---- layer 1: m-outer so each type's h' (and its gather) is
            # ready as early as possible ------------------------------------
            acc1 = [
                [pacc.tile([67, NH], F32, name=f"acc1_{i}_{hh}", tag="acc")
                 for hh in (0, 1)]
                for i in range(3)
            ]
            for m in range(6):
                i, j = PAIRS[m]
                for t2 in range(T2):
                    mv = moving_tile(m, t2, phase=0)
                    stat = xt_sb[:, j, :, t2, 0:67]
                    st = m == 2 * i and t2 == 0
                    sp = m == 2 * i + 1 and t2 == T2 - 1
                    for hh in (0, 1):
                        nc.tensor.matmul(
                            acc1[i][hh][:],
                            stat,
                            mv[:, :, hh * NH:(hh + 1) * NH],
                            start=st,
                            stop=sp,
                            perf_mode=DR,
                        )
                if m % 2 == 1:
                    # both H_ij for type i are done: linear (no bias), cast to
                    # fp8, transpose to natural, kick this type's AllGather
                    u1 = work.tile([F, R], F32, name=f"u1_{i}", tag="u")
                    hT = work.tile([F, R], BF16, name=f"hT_{i}", tag="t16")
                    for hh in (0, 1):
                        nc.vector.tensor_copy(
                            u1[:, hh * NH:(hh + 1) * NH], acc1[i][hh][0:F, :]
                        )
                        nc.vector.tensor_copy(
                            s_all[F:67, i, hh * NH:(hh + 1) * NH],
                            acc1[i][hh][F:67, :],
                        )
                        lp = pmisc.tile([F, NH], F32, name=f"lp1_{i}_{hh}", tag="misc")
                        nc.tensor.matmul(
                            lp[:],
                            w1_sb[:, i, :],
                            u1[:, hh * NH:(hh + 1) * NH],
                            start=True,
                            stop=True,
                        )
                        nc.vector.tensor_copy(hT[:, hh * NH:(hh + 1) * NH], lp[:])
                    hnat = work.tile([128, 2, LT // 2, F], F8, name=f"hnat_{i}",
                                     tag="hnat")
                    for lt in range(LT):
                        tp = pmisc.tile([128, F], BF16, name=f"tp_{i}_{lt}",
                                        tag="misc")
                        nc.tensor.transpose(
                            tp[:], hT[:, lt * 128:(lt + 1) * 128],
                            identity[0:F, 0:F],
                        )
                        nc.vector.tensor_copy(hnat[:, lt & 1, lt >> 1, :], tp[:])
                    for k in (0, 1):
                        nc.sync.dma_start(ag_in[i][k], hnat[:, k])
                    nc.gpsimd.collective_compute(
                        "AllGather",
                        mybir.AluOpType.bypass,
                        replica_groups=[list(range(CORES))],
                        ins=[ag_in[i][:]],
                        outs=[ag_out[i][:]],
                    )
                    for rank in range(CORES):
                        nc.sync.dma_start(
                            S[:, i, :, rank * 3:(rank + 1) * 3, :],
                            ag_out[i][rank].rearrange("k p t f -> p k t f"),
                        )

            # ---- layer 2: one phase per gathered source type j -------------
            acc2 = [
                [pacc.tile([F, NH], F32, name=f"acc2_{i}_{hh}", tag="acc")
                 for hh in (0, 1)]
                for i in range(3)
            ]

            def linear2(i):
                u2 = work.tile([67, R], F32, name=f"u2_{i}", tag="u")
                for hh in (0, 1):
                    nc.vector.tensor_copy(
                        u2[0:F, hh * NH:(hh + 1) * NH], acc2[i][hh][:]
                    )
                nc.vector.tensor_copy(u2[F:67, :], s_all[F:67, i, :])
                od = work.tile([F, R], BF16, name=f"od_{i}", tag="t16")
                for hh in (0, 1):
                    lp = pmisc.tile([F, NH], F32, name=f"lp2_{i}_{hh}", tag="misc")
                    nc.tensor.matmul(
                        lp[:],
                        w2_sb[0:67, i, :],
                        u2[:, hh * NH:(hh + 1) * NH],
                        start=True,
                        stop=True,
                    )
                    nc.vector.tensor_scalar_add(
                        od[:, hh * NH:(hh + 1) * NH], lp[:], b2_sb[:, i, :]
                    )
                nc.sync.dma_start(outT_d[i], od[:])

            for j in range(3):
                for t2 in range(T2):
                    stat = S[:, j, :, t2, :]
                    for m in PHASE_MS[j]:
                        i = PAIRS[m][0]
                        st = j == L2_JS[i][0] and t2 == 0
                        sp = j == L2_JS[i][1] and t2 == T2 - 1
                        mv = moving_tile(m, t2, phase=1)
                        for hh in (0, 1):
                            nc.tensor.matmul(
                                acc2[i][hh][:],
                                stat,
                                mv[:, :, hh * NH:(hh + 1) * NH],
                                start=st,
                                stop=sp,
                                perf_mode=DR,
                            )
                # types whose accumulation finished in this phase
                for i in range(3):
                    if L2_JS[i][1] == j:
                        linear2(i)

    nc.compile()
    return nc


def prep_inputs(inputs):
    """Host-side shard/transpose/cast. Returns per-core input maps."""
    fp8 = ml_dtypes.float8_e4m3

    # ht8[core, m, t2, p, k, r] = N * H_m[768*core + r, 256*t2 + 128*k + p]
    ht8_all = np.empty((CORES, 6, T2, 128, 2, R), dtype=fp8)
    for m, (i, j) in enumerate(PAIRS):
        Hm = np.asarray(inputs[f"H{i}{j}"], dtype=np.float32)
        scaled = Hm * np.float32(N)
        # [core, r, t2, k, p] -> [core, t2, p, k, r]
        perm = scaled.reshape(CORES, R, T2, 2, 128).transpose(0, 2, 4, 3, 1)
        ht8_all[:, m] = perm.astype(fp8)

    # xt8[p, j, k, t2, f] = x_j[256*t2 + 128*k + p, f]; cols 64..66 are the
    # per-type ones columns (rowsum trick), col 67 zero pad
    xt8 = np.zeros((128, 3, 2, T2, 68), dtype=fp8)
    for j in range(3):
        xj = np.asarray(inputs[f"x{j}"], dtype=np.float32)
        xt8[:, j, :, :, 0:F] = xj.reshape(T2, 2, 128, F).transpose(2, 1, 0, 3).astype(fp8)
        xt8[:, j, :, :, F + j] = np.array(1.0, dtype=fp8)

    w1t = np.ascontiguousarray(
        np.stack(
            [np.asarray(inputs[f"W1_{i}"], dtype=np.float32).T for i in range(3)],
            axis=1,
        )
    )  # [fin, 3, fout], no scale: h'_scaled = (N*u) @ W1.T

    w2e = np.zeros((67, 3, F), dtype=np.float32)
    inv_n2 = np.float32(1.0 / (float(N) * float(N)))
    inv_n = np.float32(1.0 / float(N))
    for i in range(3):
        W2 = np.asarray(inputs[f"W2_{i}"], dtype=np.float32)
        w2e[0:F, i, :] = W2.T * inv_n2
        for j in range(3):
            if j == i:
                continue
            b1j = np.asarray(inputs[f"b1_{j}"], dtype=np.float32)
            w2e[F + j, i, :] = (b1j @ W2.T) * inv_n

    b2 = np.ascontiguousarray(
        np.stack(
            [np.asarray(inputs[f"b2_{i}"], dtype=np.float32).reshape(F, 1)
             for i in range(3)],
            axis=1,
        )
    )

    shared = {"xt8": xt8, "w1t": w1t, "w2e": w2e, "b2": b2}
    return [
        {"ht8": np.ascontiguousarray(ht8_all[c]), **shared} for c in range(CORES)
    ]


_CACHED_NC = None


def get_module():
    global _CACHED_NC
    if _CACHED_NC is None:
        _CACHED_NC = build_module()
    return _CACHED_NC


def kernel(**inputs):
    import time

    nc = get_module()
    in_maps = prep_inputs(inputs)
    last_exc = None
    for attempt in range(3):
        try:
            res = run_bass_kernel_spmd(nc, in_maps, core_ids=list(range(CORES)))
            break
        except Exception as exc:  # transient NRT device errors observed on axon
            last_exc = exc
            time.sleep(5.0)
    else:
        raise last_exc
    outs = []
    for i in range(3):
        outs.append(
            np.ascontiguousarray(
                np.concatenate(
                    [res.results[c]["outT"][i].astype(np.float32).T
                     for c in range(CORES)],
                    axis=0,
                )
            )
        )
    return tuple(outs)


if __name__ == "__main__":
    rng = np.random.default_rng(0)
    inputs = {}
    for i in range(3):
        inputs[f"x{i}"] = rng.standard_normal((N, F), dtype=np.float32)
    for i, j in PAIRS:
        inputs[f"H{i}{j}"] = rng.random((N, N), dtype=np.float32) / N
    for i in range(3):
        inputs[f"W1_{i}"] = rng.standard_normal((F, F), dtype=np.float32) * 0.05
        inputs[f"b1_{i}"] = rng.standard_normal((F,), dtype=np.float32) * 0.05
        inputs[f"W2_{i}"] = rng.standard_normal((F, F), dtype=np.float32) * 0.05
        inputs[f"b2_{i}"] = rng.standard_normal((F,), dtype=np.float32) * 0.05

    out = kernel(**inputs)

    # numpy reference
    def ref(inp):
        u = [None] * 3
        u[0] = inp["H01"] @ inp["x1"] + inp["H02"] @ inp["x2"]
        u[1] = inp["H10"] @ inp["x0"] + inp["H12"] @ inp["x2"]
        u[2] = inp["H20"] @ inp["x0"] + inp["H21"] @ inp["x1"]
        h = [u[i] @ inp[f"W1_{i}"].T + inp[f"b1_{i}"] for i in range(3)]
        v = [None] * 3
        v[0] = inp["H01"] @ h[1] + inp["H02"] @ h[2]
        v[1] = inp["H10"] @ h[0] + inp["H12"] @ h[2]
        v[2] = inp["H20"] @ h[0] + inp["H21"] @ h[1]
        return tuple(v[i] @ inp[f"W2_{i}"].T + inp[f"b2_{i}"] for i in range(3))

    exp = ref(inputs)
    for i in range(3):
        a, e = out[i], exp[i]
        rel = np.abs(a - e).max() / np.abs(e).max()
        print(f"out{i}: absmax-rel err {rel:.3e}")
